# revision 1
# baseline (speedup 1.0000x reference)
"""CRF log-likelihood kernel for Trainium2 (Bass/Tile), 8-core data parallel.

out[b] = gold_path_score(b) - logZ(b)

logZ via exp-domain DP with forward and backward chains meeting at t = F:
  fwd:  u_t   = el_t  ⊙ (Wf^T u_{t-1}),      t = 1..F      (u_0 = el_0)
  bwd:  γ_σ   = Wb^T (el_{T+1-σ} ⊙ γ_{σ-1}), σ = 1..T-F    (γ_0 = sink)
Sequences with len <= F finish inside the fwd chain via an absorbing "sink"
label that captures sum_i u_{len-1}[i] exactly at t == len; longer sequences
use the midpoint identity Z = Σ_j α_F[j]·β_F[j], with the bwd chain's sink
"birthing" β = 1 at each sequence's own end time. The two chains are
independent, so PE matmuls of one overlap DVE multiplies of the other.

Layout per core (128 sequences):
  partitions 0..95 = active labels (3 groups x 32), 96..98 = sink row per
  group; psum rows 99..101 = per-group column sums (ones-columns of the
  stationary operand). columns: b_local = 43*g + c.
Scaling: all emissions carry e^{-CSHIFT}; columns are renormalized by their
column sum mid-chain (factor tracked exactly via ACT-Ln of the applied
multiplier). Host adds CSHIFT*len back and picks sink vs combine per length.
Host also does the gold-path gathers (labels/trans only) and final subtract.
"""

import numpy as np
import ml_dtypes

B, T, L = 1024, 512, 32
NCORES = 8
BPC = B // NCORES        # 128 sequences per core
G = 3                    # label groups per core
NCOL = 43                # columns per group (group 2 uses 42 + 1 pad)
NACT = 96                # active label partitions
NPART = 99               # + 3 sink rows
MOUT = 102               # + 3 colsum rows
CSHIFT = 4.5
TEX = T + 1              # el time slices 0..T
TCH = 57                 # el build chunk (9 * 57 = 513)
NCH = TEX // TCH
F = 256                  # fwd ticks; bwd ticks = T - F
SB = T - F
RENORM_EVERY = 128

_prog_cache = {}
last_result = None       # BassKernelResults of the most recent run (for test.py)


def _build_program():
    import concourse.bacc as bacc
    import concourse.tile as tile
    from concourse import mybir

    f32 = mybir.dt.float32
    bf16 = mybir.dt.bfloat16
    AF = mybir.ActivationFunctionType

    nc = bacc.Bacc("TRN2", target_bir_lowering=False, debug=False, num_devices=NCORES)
    lg = nc.dram_tensor("lg", [NACT, TEX, NCOL], f32, kind="ExternalInput")
    el32 = nc.dram_tensor("el32", [G, TEX, NCOL], bf16, kind="ExternalInput")
    wf = nc.dram_tensor("wf", [NPART, MOUT], bf16, kind="ExternalInput")
    wbk = nc.dram_tensor("wbk", [NPART, MOUT], bf16, kind="ExternalInput")
    wbc = nc.dram_tensor("wbc", [2 * G, NPART], f32, kind="ExternalInput")
    wcs = nc.dram_tensor("wcs", [NPART, G], bf16, kind="ExternalInput")
    resf = nc.dram_tensor("resf", [G, NCOL], f32, kind="ExternalOutput")
    resc = nc.dram_tensor("resc", [G, NCOL], f32, kind="ExternalOutput")

    with tile.TileContext(nc) as tc:
        with (
            tc.tile_pool(name="big", bufs=1) as big,
            tc.tile_pool(name="stage", bufs=3) as stage_p,
            tc.tile_pool(name="consts", bufs=1) as consts,
            tc.tile_pool(name="u", bufs=3) as upool,
            tc.tile_pool(name="v", bufs=3) as vpool,
            tc.tile_pool(name="small", bufs=4) as small,
            tc.tile_pool(name="fin", bufs=1) as fin,
            tc.tile_pool(name="psf", bufs=3, space="PSUM") as psfpool,
            tc.tile_pool(name="psb", bufs=3, space="PSUM") as psbpool,
            tc.tile_pool(name="psx", bufs=2, space="PSUM") as psxpool,
        ):
            el_sb = big.tile([NPART, TEX, NCOL], bf16)
            wf_sb = consts.tile([NPART, MOUT], bf16)
            wb_sb = consts.tile([NPART, MOUT], bf16)
            wbc_sb = consts.tile([2 * G, NPART], f32)
            wcs_sb = consts.tile([NPART, G], bf16)
            biasc = consts.tile([128, 1], f32)
            g0 = consts.tile([NPART, NCOL], bf16)
            nc.vector.memset(biasc[:], -CSHIFT)
            nc.vector.memset(g0[:], 0.0)
            nc.vector.memset(g0[NACT:NPART, :], 1.0)

            nc.sync.dma_start(out=wf_sb[:], in_=wf[:])
            nc.sync.dma_start(out=wb_sb[:], in_=wbk[:])
            nc.sync.dma_start(out=wbc_sb[:], in_=wbc[:])
            nc.sync.dma_start(out=wcs_sb[:], in_=wcs[:])
            # sink rows land on partitions 96..98 (one aligned DMA)
            nc.sync.dma_start(out=el_sb[NACT:NPART, :, :], in_=el32[:])
            # active rows: stage raw logits, bulk-exp into el_sb.
            # build order alternates ends: bwd consumes slices from t=T down.
            order = []
            lo, hi = 0, NCH - 1
            while lo <= hi:
                order.append(hi)
                if lo != hi:
                    order.append(lo)
                hi -= 1
                lo += 1
            for ch in order:
                st = stage_p.tile([NACT, TCH, NCOL], f32, tag="stage")
                t0 = ch * TCH
                nc.sync.dma_start(out=st[:], in_=lg[:, t0 : t0 + TCH, :])
                nc.scalar.activation(
                    el_sb[0:NACT, t0 : t0 + TCH, :], st[:], AF.Exp, bias=biasc[0:NACT, :]
                )

            lnrs_f, lnrs_b = [], []
            uprev = el_sb[:, 0, :]
            gprev = g0[:]
            gprev_sbuf = True
            ulast = None
            pb_last = None
            pend_renorm = None
            for k in range(1, max(F, SB) + 1):
                # ---- fwd tick t = k ----
                if k <= F:
                    psf = psfpool.tile([MOUT, NCOL], f32, tag="psf")
                    nc.tensor.matmul(psf[:], wf_sb[:], uprev, start=True, stop=True)
                    un = upool.tile([NPART, NCOL], bf16, tag="u")
                    nc.vector.tensor_mul(un[:], psf[0:NPART, :], el_sb[:, k, :])
                    if k % RENORM_EVERY == 0 and k < F:
                        ts6 = small.tile([2 * G, NCOL], f32, tag="ts6f")
                        nc.vector.tensor_scalar_add(
                            ts6[:], psf[NACT : NACT + 2 * G, :], 1e-30
                        )
                        rr6 = small.tile([2 * G, NCOL], f32, tag="rr6f")
                        nc.vector.reciprocal(rr6[:], ts6[:])
                        psr = psxpool.tile([NPART, NCOL], f32, tag="psr")
                        nc.tensor.matmul(psr[:], wbc_sb[:], rr6[:], start=True, stop=True)
                        un2 = upool.tile([NPART, NCOL], bf16, tag="u2")
                        nc.vector.tensor_mul(un2[:], psr[:], un[:])
                        lnr = fin.tile([G, NCOL], f32, tag=f"lnrf{len(lnrs_f)}")
                        nc.scalar.activation(lnr[:], psr[NACT:NPART, :], AF.Ln)
                        lnrs_f.append(lnr)
                        uprev = un2[:]
                    else:
                        uprev = un[:]
                    if k == F:
                        ulast = uprev
                # ---- bwd tick σ = k, el time T+1-k ----
                if k <= SB:
                    vn = vpool.tile([NPART, NCOL], bf16, tag="v")
                    src = gprev if gprev_sbuf else gprev[0:NPART, :]
                    nc.vector.tensor_mul(vn[:], src, el_sb[:, T + 1 - k, :])
                    if pend_renorm is not None:
                        # apply the deferred renorm factor (can't read two
                        # PSUM operands in one TT)
                        vn2 = vpool.tile([NPART, NCOL], bf16, tag="v2")
                        nc.vector.tensor_mul(vn2[:], pend_renorm[:], vn[:])
                        vn = vn2
                        pend_renorm = None
                    gprev_sbuf = False
                    psb = psbpool.tile([MOUT, NCOL], f32, tag="psb")
                    nc.tensor.matmul(psb[:], wb_sb[:], vn[:], start=True, stop=True)
                    if k % RENORM_EVERY == 0 and k < SB:
                        ts6b = small.tile([2 * G, NCOL], f32, tag="ts6b")
                        nc.vector.tensor_scalar_add(
                            ts6b[:], psb[NACT : NACT + 2 * G, :], 1e-30
                        )
                        rr6b = small.tile([2 * G, NCOL], f32, tag="rr6b")
                        nc.vector.reciprocal(rr6b[:], ts6b[:])
                        psrb = psxpool.tile([NPART, NCOL], f32, tag="psr")
                        nc.tensor.matmul(
                            psrb[:], wbc_sb[:], rr6b[:], start=True, stop=True
                        )
                        pend_renorm = psrb
                        lnrb = fin.tile([G, NCOL], f32, tag=f"lnrb{len(lnrs_b)}")
                        nc.scalar.activation(lnrb[:], psrb[NACT:NPART, :], AF.Ln)
                        lnrs_b.append(lnrb)
                    gprev = psb
                    if k == SB:
                        pb_last = (gprev, gprev_sbuf)

            # ---- combine: w = u_F ⊙ γ_S; Zc = per-group colsum of w ----
            gl, gl_sbuf = pb_last
            wt = vpool.tile([NPART, NCOL], bf16, tag="wt")
            nc.vector.tensor_mul(wt[:], gl if gl_sbuf else gl[0:NPART, :], ulast)
            psc = psxpool.tile([G, NCOL], f32, tag="psr")
            nc.tensor.matmul(psc[:], wcs_sb[:], wt[:], start=True, stop=True)

            # resf = ln(u_F sink) - Σ lnr_f ; resc = ln(Zc) - Σ lnr_f - Σ lnr_b
            accf = fin.tile([G, NCOL], f32, tag="lnu")
            nc.scalar.activation(accf[:], ulast[NACT:NPART, :], AF.Ln)
            for e, lnr in enumerate(lnrs_f):
                nx = fin.tile([G, NCOL], f32, tag=f"fa{e}")
                nc.vector.tensor_sub(nx[:], accf[:], lnr[:])
                accf = nx
            nc.sync.dma_start(out=resf[:], in_=accf[:])

            accc = fin.tile([G, NCOL], f32, tag="lnc")
            nc.scalar.activation(accc[:], psc[:], AF.Ln)
            for e, lnr in enumerate(lnrs_f + lnrs_b):
                nx = fin.tile([G, NCOL], f32, tag=f"ca{e}")
                nc.vector.tensor_sub(nx[:], accc[:], lnr[:])
                accc = nx
            nc.sync.dma_start(out=resc[:], in_=accc[:])

    nc.compile()
    return nc


def _host_prep(logits, trans, labels, seq_lens):
    logits = np.ascontiguousarray(np.asarray(logits), dtype=np.float32)
    trans = np.asarray(trans, dtype=np.float32)
    labels = np.asarray(labels)
    lens = np.clip(np.asarray(seq_lens), 1, T).astype(np.int64)

    # ---- gold path score (host: index gathers over small inputs) ----
    tmask = np.arange(T)[None, :] < lens[:, None]
    unary = np.take_along_axis(logits, labels[..., None].astype(np.int64), axis=2)[..., 0]
    gp = (unary * tmask).sum(1) + (trans[labels[:, :-1], labels[:, 1:]] * tmask[:, 1:]).sum(1)

    # ---- device inputs: mask every t >= len; pad slice t=T = -inf ----
    lgx = logits.copy()
    lgx[~tmask] = -1e9
    lgx = np.concatenate([lgx, np.full((B, 1, L), -1e9, np.float32)], axis=1)

    el32 = (np.arange(TEX)[None, :] >= lens[:, None]).astype(np.float32)  # [B, 513]

    lg_cores, el32_cores = [], []
    for core in range(NCORES):
        b0 = core * BPC
        lgp = np.full((G, 32, TEX, NCOL), -1e9, np.float32)
        e32 = np.zeros((G, TEX, NCOL), np.float32)
        for g in range(G):
            ncols = NCOL if g < 2 else BPC - 2 * NCOL
            bs = b0 + g * NCOL
            lgp[g, :, :, :ncols] = lgx[bs : bs + ncols].transpose(2, 1, 0)
            e32[g, :, :ncols] = el32[bs : bs + ncols].T
            if ncols < NCOL:  # pad column: dummy len==T sequence, active el = 0
                e32[g, T, ncols:] = 1.0
        lg_cores.append(np.ascontiguousarray(lgp).reshape(NACT, TEX, NCOL))
        el32_cores.append(e32.astype(ml_dtypes.bfloat16))

    # ---- stationary operators ----
    E = np.exp(trans).astype(np.float32)
    Wf = np.zeros((NPART, MOUT), np.float32)
    Wb = np.zeros((NPART, MOUT), np.float32)
    Wbc = np.zeros((2 * G, NPART), np.float32)
    Wcs = np.zeros((NPART, G), np.float32)
    for g in range(G):
        a, sk, cs = 32 * g, NACT + g, NPART + g
        Wf[a : a + 32, a : a + 32] = E
        Wf[a : a + 32, sk] = 1.0
        Wf[sk, sk] = 1.0
        Wf[a : a + 32, cs] = 1.0
        Wf[sk, cs] = 1.0
        Wb[a : a + 32, a : a + 32] = E.T
        Wb[sk, a : a + 32] = 1.0   # sink births β = 1 over all labels
        Wb[sk, sk] = 1.0
        Wb[a : a + 32, cs] = 1.0
        Wb[sk, cs] = 1.0
        Wbc[G + g, a : a + 32] = 1.0
        Wbc[G + g, sk] = 1.0
        Wcs[a : a + 32, g] = 1.0
        Wcs[sk, g] = 1.0
    bf = ml_dtypes.bfloat16
    return gp, lens, lg_cores, el32_cores, Wf.astype(bf), Wb.astype(bf), Wbc, Wcs.astype(bf)


def _log(msg):
    import time as _t

    print(f"[kernel {_t.strftime('%H:%M:%S')}] {msg}", flush=True)


def kernel(logits, trans, labels, seq_lens):
    global last_result
    from concourse.bass_utils import run_bass_kernel_spmd

    _log("host prep start")
    gp, lens, lg_cores, el32_cores, Wf, Wb, Wbc, Wcs = _host_prep(
        logits, trans, labels, seq_lens
    )
    _log("host prep done")

    if "nc" not in _prog_cache:
        _prog_cache["nc"] = _build_program()
        _log("program built")
    nc = _prog_cache["nc"]

    in_maps = [
        {
            "lg": lg_cores[i],
            "el32": el32_cores[i],
            "wf": Wf,
            "wbk": Wb,
            "wbc": Wbc,
            "wcs": Wcs,
        }
        for i in range(NCORES)
    ]
    r = run_bass_kernel_spmd(nc, in_maps, core_ids=list(range(NCORES)))
    last_result = r
    _log("device run done")

    # ---- unshard + select sink vs combine per sequence length ----
    devf = np.zeros(B, np.float32)
    devc = np.zeros(B, np.float32)
    for core in range(NCORES):
        rf = r.results[core]["resf"]
        rc = r.results[core]["resc"]
        b0 = core * BPC
        for g in range(G):
            ncols = NCOL if g < 2 else BPC - 2 * NCOL
            devf[b0 + g * NCOL : b0 + g * NCOL + ncols] = rf[g, :ncols]
            devc[b0 + g * NCOL : b0 + g * NCOL + ncols] = rc[g, :ncols]

    dev = np.where(lens <= F, devf, devc)
    logZ = dev + CSHIFT * lens.astype(np.float32)
    return (gp - logZ).astype(np.float32)



# revision 6
# speedup vs baseline: 2.2750x; 2.2750x over previous
"""CRF log-likelihood kernel for Trainium2 (Bass/Tile), 8-core data parallel.

out[b] = gold_path_score(b) - logZ(b)

logZ via K=16 parallel forward chains in the exp domain. Chain k owns
el-times (32k, 32k+32] and starts BURN=8 ticks early at a_k = 32k-8 with an
arbitrary positive init (the el slice at a_k). Products of positive matrices
contract in the Hilbert projective metric (Birkhoff coefficient of
E = exp(trans) is <= tanh(1/2) ~ 0.46 per step; the diagonal emission scaling
is a projective isometry), so after the burn-in each chain's state is
proportional to the true alpha: y^k_t = lam_k * alpha_t. The scale factors
cancel via per-sequence telescoping of the linear functional f_t = 1^T u_t
read at the shared boundary el-time 32k-1 from both neighbouring chains:

  log lam_k = log lam_{k-1} + log f(y^k) - log f(y^{k-1})   (same t, same f)

Each sequence's logZ is read from the sink state of the chain containing its
end position: sink captures 1^T u_{len-1} exactly at t == len and is
absorbing afterwards. Serial depth drops from 256 ticks (fwd+bwd midpoint
baseline) to NT = 40 ticks.

Layout per core (128 sequences x 16 chains = 2048 chain-columns):
  2 strands x 3 groups x 342 columns; partitions 0..95 = active labels
  (3 groups x 32), 96..98 = sink row per group; psum rows 99..101 =
  per-group column sums (ones-columns of the stationary operand).
  Strand 0 = chains 0..7 (TT on Vector), strand 1 = chains 8..15 (TT on
  GpSimd), so the two serial MM->TT->MM dependency chains overlap across
  engines. Snapshots (ACT-Ln of psum colsum rows) at ticks 8/32/40; final
  sink rows Ln'd from the last state. All emissions carry e^{-CSHIFT}; host
  adds CSHIFT*len back and does the gold-path gathers and telescoping.
"""

import numpy as np
import ml_dtypes

B, T, L = 1024, 512, 32
NCORES = 8
BPC = B // NCORES        # 128 sequences per core
K = 16                   # parallel chains per sequence
SEG = T // K             # 32 el-times owned per chain
BURN = 8                 # burn-in ticks (direction convergence)
NT = BURN + SEG          # 40 ticks per chain
NS = 2                   # strands (independent MM->TT dependency chains)
G = 3                    # label groups per strand
NCOL = 342               # columns per group (1024 pairs = 3*342 - 2)
PPS = K * BPC // NS      # 1024 (k,b) pairs per strand
NACT = 96
NPART = 99
MOUT = 102
CSHIFT = 4.5
TAU_SNAP = (BURN, SEG, NT)   # colsum snapshot ticks

_prog_cache = {}
last_result = None       # BassKernelResults of the most recent run (for test.py)


def _build_program():
    import concourse.bacc as bacc
    import concourse.tile as tile
    from concourse import mybir

    f32 = mybir.dt.float32
    bf16 = mybir.dt.bfloat16
    AF = mybir.ActivationFunctionType

    nc = bacc.Bacc("TRN2", target_bir_lowering=False, debug=False, num_devices=NCORES)
    els = [
        nc.dram_tensor(f"el{s}", [NPART, NT + 1, NCOL], bf16, kind="ExternalInput")
        for s in range(NS)
    ]
    wf = nc.dram_tensor("wf", [NPART, MOUT], bf16, kind="ExternalInput")
    outs = [
        nc.dram_tensor(f"res{s}", [21, NCOL], f32, kind="ExternalOutput")
        for s in range(NS)
    ]

    CH = 8  # el DMA chunk: 8 tick-slices

    with tile.TileContext(nc) as tc:
        with (
            tc.tile_pool(name="big", bufs=1) as big,
            tc.tile_pool(name="consts", bufs=1) as consts,
            tc.tile_pool(name="u0", bufs=3) as up0,
            tc.tile_pool(name="u1", bufs=3) as up1,
            tc.tile_pool(name="fin", bufs=1) as fin,
            tc.tile_pool(name="ps0", bufs=3, space="PSUM") as psp0,
            tc.tile_pool(name="ps1", bufs=3, space="PSUM") as psp1,
        ):
            wf_sb = consts.tile([NPART, MOUT], bf16)
            nc.sync.dma_start(out=wf_sb[:], in_=wf[:])

            el_sb = [big.tile([NPART, NT + 1, NCOL], bf16, tag=f"el{s}", name=f"el_sb{s}") for s in range(NS)]
            t0 = 0
            while t0 < NT + 1:
                t1 = min(t0 + CH, NT + 1)
                for s in range(NS):
                    nc.sync.dma_start(out=el_sb[s][:, t0:t1, :], in_=els[s][:, t0:t1, :])
                t0 = t1

            snaps = [
                [fin.tile([6, NCOL], f32, tag=f"sn{s}{j}", name=f"sn{s}{j}") for j in range(3)]
                for s in range(NS)
            ]
            snks = [fin.tile([3, NCOL], f32, tag=f"snk{s}", name=f"snk{s}") for s in range(NS)]
            upools = (up0, up1)
            pspools = (psp0, psp1)
            tt_eng = (nc.vector, nc.vector)

            u = [el_sb[s][:, 0, :] for s in range(NS)]
            for tau in range(1, NT + 1):
                for s in range(NS):
                    ps = pspools[s].tile([MOUT, NCOL], f32, tag=f"ps{s}", name=f"ps{s}")
                    nc.tensor.matmul(ps[:], wf_sb[:], u[s], start=True, stop=True)
                    if tau in TAU_SNAP:
                        j = TAU_SNAP.index(tau)
                        nc.scalar.activation(snaps[s][j][:], ps[NACT:MOUT, :], AF.Ln)
                    un = upools[s].tile([NPART, NCOL], bf16, tag=f"u{s}", name=f"un{s}")
                    tt_eng[s].tensor_mul(un[:], ps[0:NPART, :], el_sb[s][:, tau, :])
                    u[s] = un[:]

            for s in range(NS):
                nc.scalar.activation(snks[s][:], u[s][NACT:NPART, :], AF.Ln)
                for j in range(3):
                    nc.sync.dma_start(out=outs[s][6 * j : 6 * j + 6, :], in_=snaps[s][j][:])
                nc.sync.dma_start(out=outs[s][18:21, :], in_=snks[s][:])

    nc.compile()
    return nc


def _host_prep(logits, trans, labels, seq_lens):
    logits = np.ascontiguousarray(np.asarray(logits), dtype=np.float32)
    trans = np.asarray(trans, dtype=np.float32)
    labels = np.asarray(labels)
    lens = np.clip(np.asarray(seq_lens), 1, T).astype(np.int64)

    # ---- gold path score (host: index gathers over small inputs) ----
    tmask = np.arange(T)[None, :] < lens[:, None]
    unary = np.take_along_axis(logits, labels[..., None].astype(np.int64), axis=2)[..., 0]
    gp = (unary * tmask).sum(1) + (trans[labels[:, :-1], labels[:, 1:]] * tmask[:, 1:]).sum(1)

    # ---- full emission tables over el-time 0..T ----
    act = np.where(tmask[:, :, None], np.exp(logits - CSHIFT), 0.0).astype(np.float32)
    act = np.concatenate([act, np.zeros((B, 1, L), np.float32)], axis=1)  # [B,513,L]
    snk = (np.arange(T + 1)[None, :] >= lens[:, None]).astype(np.float32)  # [B,513]

    # chain start offsets and per-tick el-times
    a_k = np.maximum(np.arange(K) * SEG - BURN, 0)           # [K]
    times = a_k[:, None] + np.arange(NT + 1)[None, :]        # [K, NT+1]

    bf = ml_dtypes.bfloat16
    el_cores = []
    for core in range(NCORES):
        b0 = core * BPC
        # pair p = k*BPC + b_local ; strand = p // PPS ; idx = p % PPS
        bidx = b0 + np.tile(np.arange(BPC), K).reshape(K, BPC)       # [K,B_l]
        ap = act[bidx[:, :, None], times[:, None, :], :]             # [K,B_l,NT+1,L]
        sp = snk[bidx[:, :, None], times[:, None, :]]                # [K,B_l,NT+1]
        ap = ap.reshape(K * BPC, NT + 1, L)
        sp = sp.reshape(K * BPC, NT + 1)
        per_strand = []
        for s in range(NS):
            a_s = ap[s * PPS : (s + 1) * PPS]                        # [1024,NT+1,L]
            s_s = sp[s * PPS : (s + 1) * PPS]
            buf = np.zeros((G, 32, NT + 1, NCOL), np.float32)
            sbuf = np.zeros((G, NT + 1, NCOL), np.float32)
            for g in range(G):
                i0 = g * NCOL
                ncols = min(NCOL, PPS - i0)
                buf[g, :, :, :ncols] = a_s[i0 : i0 + ncols].transpose(2, 1, 0)
                sbuf[g, :, :ncols] = s_s[i0 : i0 + ncols].T
                if ncols < NCOL:
                    sbuf[g, :, ncols:] = 1.0  # pad: pure-sink column
            full = np.concatenate([buf.reshape(NACT, NT + 1, NCOL), sbuf], axis=0)
            per_strand.append(np.ascontiguousarray(full.astype(bf)))
        el_cores.append(per_strand)

    # ---- stationary operator: E blocks + sink + colsum columns ----
    E = np.exp(trans).astype(np.float32)
    Wf = np.zeros((NPART, MOUT), np.float32)
    for g in range(G):
        a, sk, cs = 32 * g, NACT + g, NPART + g
        Wf[a : a + 32, a : a + 32] = E
        Wf[a : a + 32, sk] = 1.0
        Wf[sk, sk] = 1.0
        Wf[a : a + 32, cs] = 1.0
        Wf[sk, cs] = 1.0
    return gp, lens, el_cores, Wf.astype(bf)


def _log(msg):
    import time as _t

    print(f"[kernel {_t.strftime('%H:%M:%S')}] {msg}", flush=True)


def kernel(logits, trans, labels, seq_lens):
    global last_result
    from concourse.bass_utils import run_bass_kernel_spmd

    _log("host prep start")
    gp, lens, el_cores, Wf = _host_prep(logits, trans, labels, seq_lens)
    _log("host prep done")

    if "nc" not in _prog_cache:
        _prog_cache["nc"] = _build_program()
        _log("program built")
    nc = _prog_cache["nc"]

    in_maps = [
        {"wf": Wf, **{f"el{s}": el_cores[i][s] for s in range(NS)}}
        for i in range(NCORES)
    ]
    r = run_bass_kernel_spmd(nc, in_maps, core_ids=list(range(NCORES)))
    last_result = r
    _log("device run done")

    # ---- unshard: per (k, b) snapshots and sink reads ----
    lnS = np.zeros((K, B), np.float64)   # colsum at tick BURN   (chain start)
    lnM = np.zeros((K, B), np.float64)   # colsum at tick SEG    (chain-0 end)
    lnE = np.zeros((K, B), np.float64)   # colsum at tick NT     (chain end)
    lnK = np.zeros((K, B), np.float64)   # final sink rows
    for core in range(NCORES):
        b0 = core * BPC
        for s in range(NS):
            res = r.results[core][f"res{s}"].astype(np.float64)  # [21, NCOL]
            idx = np.arange(PPS)
            g, c = idx // NCOL, idx % NCOL
            p = s * PPS + idx
            k, bl = p // BPC, p % BPC
            lnS[k, b0 + bl] = res[3 + g, c]
            lnM[k, b0 + bl] = res[9 + g, c]
            lnE[k, b0 + bl] = res[15 + g, c]
            lnK[k, b0 + bl] = res[18 + g, c]

    # ---- telescoping: log lam_k relative to the exact chain 0 ----
    loglam = np.zeros((K, B), np.float64)
    prev_end = lnM[0]                      # chain 0 boundary read at tick SEG
    for k in range(1, K):
        loglam[k] = loglam[k - 1] + lnS[k] - prev_end
        prev_end = lnE[k]

    kb = np.clip((lens - 1) // SEG, 0, K - 1)
    ar = np.arange(B)
    logZ = lnK[kb, ar] - loglam[kb, ar] + CSHIFT * lens.astype(np.float64)
    return (gp - logZ).astype(np.float32)


# revision 7
# speedup vs baseline: 2.7434x; 1.2059x over previous
"""CRF log-likelihood kernel for Trainium2 (Bass/Tile), 8-core data parallel.

out[b] = gold_path_score(b) - logZ(b)

logZ via K=16 parallel forward chains in the exp domain. Chain k owns
el-times (32k, 32k+32] and starts BURN=8 ticks early at a_k = 32k-8 with an
arbitrary positive init (the el slice at a_k). Products of positive matrices
contract in the Hilbert projective metric (Birkhoff coefficient of
E = exp(trans) is <= tanh(1/2) ~ 0.46 per step; the diagonal emission scaling
is a projective isometry), so after the burn-in each chain's state is
proportional to the true alpha: y^k_t = lam_k * alpha_t. The scale factors
cancel via per-sequence telescoping of the linear functional f_t = 1^T u_t
read at the shared boundary el-time 32k-1 from both neighbouring chains:

  log lam_k = log lam_{k-1} + log f(y^k) - log f(y^{k-1})   (same t, same f)

Each sequence's logZ is read from the sink state of the chain containing its
end position: sink captures 1^T u_{len-1} exactly at t == len and is
absorbing afterwards. Serial depth drops from 256 ticks (fwd+bwd midpoint
baseline) to NT = 40 ticks.

Layout per core (128 sequences x 16 chains = 2048 chain-columns):
  2 strands x 3 groups x 342 columns; partitions 0..95 = active labels
  (3 groups x 32), 96..98 = sink row per group; psum rows 99..101 =
  per-group column sums (ones-columns of the stationary operand).
  Strand 0 = chains 0..7 (TT on Vector), strand 1 = chains 8..15 (TT on
  GpSimd), so the two serial MM->TT->MM dependency chains overlap across
  engines. Snapshots (ACT-Ln of psum colsum rows) at ticks 8/32/40; final
  sink rows Ln'd from the last state. All emissions carry e^{-CSHIFT}; host
  adds CSHIFT*len back and does the gold-path gathers and telescoping.
"""

import numpy as np
import ml_dtypes

B, T, L = 1024, 512, 32
NCORES = 8
BPC = B // NCORES        # 128 sequences per core
K = 32                   # parallel chains per sequence
SEG = T // K             # el-times owned per chain
BURN = 4                 # burn-in ticks (direction convergence)
NT = BURN + SEG          # ticks per chain
NS = 4                   # strands (independent MM->TT dependency chains)
G = 3                    # label groups per strand
NCOL = 342               # columns per group (1024 pairs = 3*342 - 2)
PPS = K * BPC // NS      # 1024 (k,b) pairs per strand
NACT = 96
NPART = 99
MOUT = 102
CSHIFT = 4.5
TAU_SNAP = (BURN, SEG, NT)   # colsum snapshot ticks

_prog_cache = {}
last_result = None       # BassKernelResults of the most recent run (for test.py)


def _build_program():
    import concourse.bacc as bacc
    import concourse.tile as tile
    from concourse import mybir

    f32 = mybir.dt.float32
    bf16 = mybir.dt.bfloat16
    AF = mybir.ActivationFunctionType

    nc = bacc.Bacc("TRN2", target_bir_lowering=False, debug=False, num_devices=NCORES)
    els = [
        nc.dram_tensor(f"el{s}", [NPART, NT + 1, NCOL], bf16, kind="ExternalInput")
        for s in range(NS)
    ]
    wf = nc.dram_tensor("wf", [NPART, MOUT], bf16, kind="ExternalInput")
    outs = [
        nc.dram_tensor(f"res{s}", [21, NCOL], f32, kind="ExternalOutput")
        for s in range(NS)
    ]

    # el DMA chunk boundaries: small first chunk so tick 1 starts early
    CUTS = [0, 2, 6, 11, 16, NT + 1]

    with tile.TileContext(nc) as tc:
        with (
            tc.tile_pool(name="big", bufs=1) as big,
            tc.tile_pool(name="consts", bufs=1) as consts,
            tc.tile_pool(name="u0", bufs=3) as up0,
            tc.tile_pool(name="u1", bufs=3) as up1,
            tc.tile_pool(name="u2", bufs=3) as up2,
            tc.tile_pool(name="u3", bufs=3) as up3,
            tc.tile_pool(name="fin", bufs=1) as fin,
            tc.tile_pool(name="ps0", bufs=2, space="PSUM") as psp0,
            tc.tile_pool(name="ps1", bufs=2, space="PSUM") as psp1,
            tc.tile_pool(name="ps2", bufs=2, space="PSUM") as psp2,
            tc.tile_pool(name="ps3", bufs=2, space="PSUM") as psp3,
        ):
            wf_sb = consts.tile([NPART, MOUT], bf16)
            nc.sync.dma_start(out=wf_sb[:], in_=wf[:])

            el_sb = [big.tile([NPART, NT + 1, NCOL], bf16, tag=f"el{s}", name=f"el_sb{s}") for s in range(NS)]
            for ci in range(len(CUTS) - 1):
                t0, t1 = CUTS[ci], CUTS[ci + 1]
                for s in range(NS):
                    eng = nc.sync if s % 2 == 0 else nc.scalar
                    eng.dma_start(out=el_sb[s][:, t0:t1, :], in_=els[s][:, t0:t1, :])

            snaps = [
                [fin.tile([6, NCOL], f32, tag=f"sn{s}{j}", name=f"sn{s}{j}") for j in range(3)]
                for s in range(NS)
            ]
            snks = [fin.tile([3, NCOL], f32, tag=f"snk{s}", name=f"snk{s}") for s in range(NS)]
            upools = (up0, up1, up2, up3)
            pspools = (psp0, psp1, psp2, psp3)
            tt_eng = (nc.vector,) * NS

            u = [el_sb[s][:, 0, :] for s in range(NS)]
            for tau in range(1, NT + 1):
                for s in range(NS):
                    ps = pspools[s].tile([MOUT, NCOL], f32, tag=f"ps{s}", name=f"ps{s}")
                    nc.tensor.matmul(ps[:], wf_sb[:], u[s], start=True, stop=True)
                    if tau in TAU_SNAP:
                        j = TAU_SNAP.index(tau)
                        nc.scalar.activation(snaps[s][j][:], ps[NACT:MOUT, :], AF.Ln)
                    un = upools[s].tile([NPART, NCOL], bf16, tag=f"u{s}", name=f"un{s}")
                    tt_eng[s].tensor_mul(un[:], ps[0:NPART, :], el_sb[s][:, tau, :])
                    u[s] = un[:]

            for s in range(NS):
                nc.scalar.activation(snks[s][:], u[s][NACT:NPART, :], AF.Ln)
                for j in range(3):
                    nc.sync.dma_start(out=outs[s][6 * j : 6 * j + 6, :], in_=snaps[s][j][:])
                nc.sync.dma_start(out=outs[s][18:21, :], in_=snks[s][:])

    nc.compile()
    return nc


def _host_prep(logits, trans, labels, seq_lens):
    logits = np.ascontiguousarray(np.asarray(logits), dtype=np.float32)
    trans = np.asarray(trans, dtype=np.float32)
    labels = np.asarray(labels)
    lens = np.clip(np.asarray(seq_lens), 1, T).astype(np.int64)

    # ---- gold path score (host: index gathers over small inputs) ----
    tmask = np.arange(T)[None, :] < lens[:, None]
    unary = np.take_along_axis(logits, labels[..., None].astype(np.int64), axis=2)[..., 0]
    gp = (unary * tmask).sum(1) + (trans[labels[:, :-1], labels[:, 1:]] * tmask[:, 1:]).sum(1)

    # ---- full emission tables over el-time 0..T ----
    act = np.where(tmask[:, :, None], np.exp(logits - CSHIFT), 0.0).astype(np.float32)
    act = np.concatenate([act, np.zeros((B, 1, L), np.float32)], axis=1)  # [B,513,L]
    snk = (np.arange(T + 1)[None, :] >= lens[:, None]).astype(np.float32)  # [B,513]

    # chain start offsets and per-tick el-times
    a_k = np.maximum(np.arange(K) * SEG - BURN, 0)           # [K]
    times = a_k[:, None] + np.arange(NT + 1)[None, :]        # [K, NT+1]

    bf = ml_dtypes.bfloat16
    el_cores = []
    for core in range(NCORES):
        b0 = core * BPC
        # pair p = k*BPC + b_local ; strand = p // PPS ; idx = p % PPS
        bidx = b0 + np.tile(np.arange(BPC), K).reshape(K, BPC)       # [K,B_l]
        ap = act[bidx[:, :, None], times[:, None, :], :]             # [K,B_l,NT+1,L]
        sp = snk[bidx[:, :, None], times[:, None, :]]                # [K,B_l,NT+1]
        ap = ap.reshape(K * BPC, NT + 1, L)
        sp = sp.reshape(K * BPC, NT + 1)
        per_strand = []
        for s in range(NS):
            a_s = ap[s * PPS : (s + 1) * PPS]                        # [1024,NT+1,L]
            s_s = sp[s * PPS : (s + 1) * PPS]
            buf = np.zeros((G, 32, NT + 1, NCOL), np.float32)
            sbuf = np.zeros((G, NT + 1, NCOL), np.float32)
            for g in range(G):
                i0 = g * NCOL
                ncols = min(NCOL, PPS - i0)
                buf[g, :, :, :ncols] = a_s[i0 : i0 + ncols].transpose(2, 1, 0)
                sbuf[g, :, :ncols] = s_s[i0 : i0 + ncols].T
                if ncols < NCOL:
                    sbuf[g, :, ncols:] = 1.0  # pad: pure-sink column
            full = np.concatenate([buf.reshape(NACT, NT + 1, NCOL), sbuf], axis=0)
            per_strand.append(np.ascontiguousarray(full.astype(bf)))
        el_cores.append(per_strand)

    # ---- stationary operator: E blocks + sink + colsum columns ----
    E = np.exp(trans).astype(np.float32)
    Wf = np.zeros((NPART, MOUT), np.float32)
    for g in range(G):
        a, sk, cs = 32 * g, NACT + g, NPART + g
        Wf[a : a + 32, a : a + 32] = E
        Wf[a : a + 32, sk] = 1.0
        Wf[sk, sk] = 1.0
        Wf[a : a + 32, cs] = 1.0
        Wf[sk, cs] = 1.0
    return gp, lens, el_cores, Wf.astype(bf)


def _log(msg):
    import time as _t

    print(f"[kernel {_t.strftime('%H:%M:%S')}] {msg}", flush=True)


def kernel(logits, trans, labels, seq_lens):
    global last_result
    from concourse.bass_utils import run_bass_kernel_spmd

    _log("host prep start")
    gp, lens, el_cores, Wf = _host_prep(logits, trans, labels, seq_lens)
    _log("host prep done")

    if "nc" not in _prog_cache:
        _prog_cache["nc"] = _build_program()
        _log("program built")
    nc = _prog_cache["nc"]

    in_maps = [
        {"wf": Wf, **{f"el{s}": el_cores[i][s] for s in range(NS)}}
        for i in range(NCORES)
    ]
    r = run_bass_kernel_spmd(nc, in_maps, core_ids=list(range(NCORES)))
    last_result = r
    _log("device run done")

    # ---- unshard: per (k, b) snapshots and sink reads ----
    lnS = np.zeros((K, B), np.float64)   # colsum at tick BURN   (chain start)
    lnM = np.zeros((K, B), np.float64)   # colsum at tick SEG    (chain-0 end)
    lnE = np.zeros((K, B), np.float64)   # colsum at tick NT     (chain end)
    lnK = np.zeros((K, B), np.float64)   # final sink rows
    for core in range(NCORES):
        b0 = core * BPC
        for s in range(NS):
            res = r.results[core][f"res{s}"].astype(np.float64)  # [21, NCOL]
            idx = np.arange(PPS)
            g, c = idx // NCOL, idx % NCOL
            p = s * PPS + idx
            k, bl = p // BPC, p % BPC
            lnS[k, b0 + bl] = res[3 + g, c]
            lnM[k, b0 + bl] = res[9 + g, c]
            lnE[k, b0 + bl] = res[15 + g, c]
            lnK[k, b0 + bl] = res[18 + g, c]

    # ---- telescoping: log lam_k relative to the exact chain 0 ----
    loglam = np.zeros((K, B), np.float64)
    prev_end = lnM[0]                      # chain 0 boundary read at tick SEG
    for k in range(1, K):
        loglam[k] = loglam[k - 1] + lnS[k] - prev_end
        prev_end = lnE[k]

    kb = np.clip((lens - 1) // SEG, 0, K - 1)
    ar = np.arange(B)
    logZ = lnK[kb, ar] - loglam[kb, ar] + CSHIFT * lens.astype(np.float64)
    return (gp - logZ).astype(np.float32)


# revision 9
# speedup vs baseline: 3.1585x; 1.1513x over previous
"""CRF log-likelihood kernel for Trainium2 (Bass/Tile), 8-core data parallel.

out[b] = gold_path_score(b) - logZ(b)

logZ via K=16 parallel forward chains in the exp domain. Chain k owns
el-times (32k, 32k+32] and starts BURN=8 ticks early at a_k = 32k-8 with an
arbitrary positive init (the el slice at a_k). Products of positive matrices
contract in the Hilbert projective metric (Birkhoff coefficient of
E = exp(trans) is <= tanh(1/2) ~ 0.46 per step; the diagonal emission scaling
is a projective isometry), so after the burn-in each chain's state is
proportional to the true alpha: y^k_t = lam_k * alpha_t. The scale factors
cancel via per-sequence telescoping of the linear functional f_t = 1^T u_t
read at the shared boundary el-time 32k-1 from both neighbouring chains:

  log lam_k = log lam_{k-1} + log f(y^k) - log f(y^{k-1})   (same t, same f)

Each sequence's logZ is read from the sink state of the chain containing its
end position: sink captures 1^T u_{len-1} exactly at t == len and is
absorbing afterwards. Serial depth drops from 256 ticks (fwd+bwd midpoint
baseline) to NT = 40 ticks.

Layout per core (128 sequences x 16 chains = 2048 chain-columns):
  2 strands x 3 groups x 342 columns; partitions 0..95 = active labels
  (3 groups x 32), 96..98 = sink row per group; psum rows 99..101 =
  per-group column sums (ones-columns of the stationary operand).
  Strand 0 = chains 0..7 (TT on Vector), strand 1 = chains 8..15 (TT on
  GpSimd), so the two serial MM->TT->MM dependency chains overlap across
  engines. Snapshots (ACT-Ln of psum colsum rows) at ticks 8/32/40; final
  sink rows Ln'd from the last state. All emissions carry e^{-CSHIFT}; host
  adds CSHIFT*len back and does the gold-path gathers and telescoping.
"""

import numpy as np
import ml_dtypes

B, T, L = 1024, 512, 32
NCORES = 8
BPC = B // NCORES        # 128 sequences per core
K = 32                   # parallel chains per sequence
SEG = T // K             # el-times owned per chain
BURN = 2                 # burn-in ticks (direction convergence)
NT = BURN + SEG          # ticks per chain
NS = 4                   # strands (independent MM->TT dependency chains)
G = 3                    # label groups per strand
NCOL = 342               # columns per group (1024 pairs = 3*342 - 2)
PPS = K * BPC // NS      # 1024 (k,b) pairs per strand
NACT = 96
NPART = 99
MOUT = 102
CSHIFT = 4.5
TAU_SNAP = (BURN, SEG, NT)   # colsum snapshot ticks

_prog_cache = {}
last_result = None       # BassKernelResults of the most recent run (for test.py)


def _build_program():
    import concourse.bacc as bacc
    import concourse.tile as tile
    from concourse import mybir

    f32 = mybir.dt.float32
    bf16 = mybir.dt.bfloat16
    AF = mybir.ActivationFunctionType

    nc = bacc.Bacc("TRN2", target_bir_lowering=False, debug=False, num_devices=NCORES)
    els = [
        nc.dram_tensor(f"el{s}", [NPART, NT + 1, NCOL], bf16, kind="ExternalInput")
        for s in range(NS)
    ]
    wf = nc.dram_tensor("wf", [NPART, MOUT], bf16, kind="ExternalInput")
    outs = [
        nc.dram_tensor(f"res{s}", [21, NCOL], f32, kind="ExternalOutput")
        for s in range(NS)
    ]

    # el DMA chunk boundaries: small first chunk so tick 1 starts early
    CUTS = [0, 2, 6, 12, NT + 1]

    with tile.TileContext(nc) as tc:
        with (
            tc.tile_pool(name="big", bufs=1) as big,
            tc.tile_pool(name="consts", bufs=1) as consts,
            tc.tile_pool(name="u0", bufs=3) as up0,
            tc.tile_pool(name="u1", bufs=3) as up1,
            tc.tile_pool(name="u2", bufs=3) as up2,
            tc.tile_pool(name="u3", bufs=3) as up3,
            tc.tile_pool(name="fin", bufs=1) as fin,
            tc.tile_pool(name="ps0", bufs=2, space="PSUM") as psp0,
            tc.tile_pool(name="ps1", bufs=2, space="PSUM") as psp1,
            tc.tile_pool(name="ps2", bufs=2, space="PSUM") as psp2,
            tc.tile_pool(name="ps3", bufs=2, space="PSUM") as psp3,
        ):
            wf_sb = consts.tile([NPART, MOUT], bf16)
            nc.sync.dma_start(out=wf_sb[:], in_=wf[:])

            el_sb = [big.tile([NPART, NT + 1, NCOL], bf16, tag=f"el{s}", name=f"el_sb{s}") for s in range(NS)]
            dma_engs = (nc.sync, nc.scalar)
            di = 0
            for ci in range(len(CUTS) - 1):
                t0, t1 = CUTS[ci], CUTS[ci + 1]
                for s in range(NS):
                    dma_engs[di % 2].dma_start(
                        out=el_sb[s][:, t0:t1, :], in_=els[s][:, t0:t1, :]
                    )
                    di += 1

            snaps = [
                [fin.tile([6, NCOL], f32, tag=f"sn{s}{j}", name=f"sn{s}{j}") for j in range(3)]
                for s in range(NS)
            ]
            snks = [fin.tile([3, NCOL], f32, tag=f"snk{s}", name=f"snk{s}") for s in range(NS)]
            upools = (up0, up1, up2, up3)
            pspools = (psp0, psp1, psp2, psp3)
            tt_eng = (nc.vector,) * NS

            u = [el_sb[s][:, 0, :] for s in range(NS)]
            for tau in range(1, NT + 1):
                for s in range(NS):
                    ps = pspools[s].tile([MOUT, NCOL], f32, tag=f"ps{s}", name=f"ps{s}")
                    nc.tensor.matmul(ps[:], wf_sb[:], u[s], start=True, stop=True)
                    if tau in TAU_SNAP:
                        j = TAU_SNAP.index(tau)
                        nc.scalar.activation(snaps[s][j][:], ps[NACT:MOUT, :], AF.Ln)
                        if j < 2:
                            nc.sync.dma_start(
                                out=outs[s][6 * j : 6 * j + 6, :], in_=snaps[s][j][:]
                            )
                    un = upools[s].tile([NPART, NCOL], bf16, tag=f"u{s}", name=f"un{s}")
                    tt_eng[s].tensor_mul(un[:], ps[0:NPART, :], el_sb[s][:, tau, :])
                    u[s] = un[:]

            for s in range(NS):
                nc.scalar.activation(snks[s][:], u[s][NACT:NPART, :], AF.Ln)
                nc.sync.dma_start(out=outs[s][12:18, :], in_=snaps[s][2][:])
                nc.scalar.dma_start(out=outs[s][18:21, :], in_=snks[s][:])

    nc.compile()
    return nc


def _host_prep(logits, trans, labels, seq_lens):
    logits = np.ascontiguousarray(np.asarray(logits), dtype=np.float32)
    trans = np.asarray(trans, dtype=np.float32)
    labels = np.asarray(labels)
    lens = np.clip(np.asarray(seq_lens), 1, T).astype(np.int64)

    # ---- gold path score (host: index gathers over small inputs) ----
    tmask = np.arange(T)[None, :] < lens[:, None]
    unary = np.take_along_axis(logits, labels[..., None].astype(np.int64), axis=2)[..., 0]
    gp = (unary * tmask).sum(1) + (trans[labels[:, :-1], labels[:, 1:]] * tmask[:, 1:]).sum(1)

    # ---- full emission tables over el-time 0..T ----
    act = np.where(tmask[:, :, None], np.exp(logits - CSHIFT), 0.0).astype(np.float32)
    act = np.concatenate([act, np.zeros((B, 1, L), np.float32)], axis=1)  # [B,513,L]
    snk = (np.arange(T + 1)[None, :] >= lens[:, None]).astype(np.float32)  # [B,513]

    # chain start offsets and per-tick el-times
    a_k = np.maximum(np.arange(K) * SEG - BURN, 0)           # [K]
    times = a_k[:, None] + np.arange(NT + 1)[None, :]        # [K, NT+1]

    bf = ml_dtypes.bfloat16
    el_cores = []
    for core in range(NCORES):
        b0 = core * BPC
        # pair p = k*BPC + b_local ; strand = p // PPS ; idx = p % PPS
        bidx = b0 + np.tile(np.arange(BPC), K).reshape(K, BPC)       # [K,B_l]
        ap = act[bidx[:, :, None], times[:, None, :], :]             # [K,B_l,NT+1,L]
        sp = snk[bidx[:, :, None], times[:, None, :]]                # [K,B_l,NT+1]
        ap = ap.reshape(K * BPC, NT + 1, L)
        sp = sp.reshape(K * BPC, NT + 1)
        per_strand = []
        for s in range(NS):
            a_s = ap[s * PPS : (s + 1) * PPS]                        # [1024,NT+1,L]
            s_s = sp[s * PPS : (s + 1) * PPS]
            buf = np.zeros((G, 32, NT + 1, NCOL), np.float32)
            sbuf = np.zeros((G, NT + 1, NCOL), np.float32)
            for g in range(G):
                i0 = g * NCOL
                ncols = min(NCOL, PPS - i0)
                buf[g, :, :, :ncols] = a_s[i0 : i0 + ncols].transpose(2, 1, 0)
                sbuf[g, :, :ncols] = s_s[i0 : i0 + ncols].T
                if ncols < NCOL:
                    sbuf[g, :, ncols:] = 1.0  # pad: pure-sink column
            full = np.concatenate([buf.reshape(NACT, NT + 1, NCOL), sbuf], axis=0)
            per_strand.append(np.ascontiguousarray(full.astype(bf)))
        el_cores.append(per_strand)

    # ---- stationary operator: E blocks + sink + colsum columns ----
    E = np.exp(trans).astype(np.float32)
    Wf = np.zeros((NPART, MOUT), np.float32)
    for g in range(G):
        a, sk, cs = 32 * g, NACT + g, NPART + g
        Wf[a : a + 32, a : a + 32] = E
        Wf[a : a + 32, sk] = 1.0
        Wf[sk, sk] = 1.0
        Wf[a : a + 32, cs] = 1.0
        Wf[sk, cs] = 1.0
    return gp, lens, el_cores, Wf.astype(bf)


def _log(msg):
    import time as _t

    print(f"[kernel {_t.strftime('%H:%M:%S')}] {msg}", flush=True)


def kernel(logits, trans, labels, seq_lens):
    global last_result
    from concourse.bass_utils import run_bass_kernel_spmd

    _log("host prep start")
    gp, lens, el_cores, Wf = _host_prep(logits, trans, labels, seq_lens)
    _log("host prep done")

    if "nc" not in _prog_cache:
        _prog_cache["nc"] = _build_program()
        _log("program built")
    nc = _prog_cache["nc"]

    in_maps = [
        {"wf": Wf, **{f"el{s}": el_cores[i][s] for s in range(NS)}}
        for i in range(NCORES)
    ]
    r = run_bass_kernel_spmd(nc, in_maps, core_ids=list(range(NCORES)))
    last_result = r
    _log("device run done")

    # ---- unshard: per (k, b) snapshots and sink reads ----
    lnS = np.zeros((K, B), np.float64)   # colsum at tick BURN   (chain start)
    lnM = np.zeros((K, B), np.float64)   # colsum at tick SEG    (chain-0 end)
    lnE = np.zeros((K, B), np.float64)   # colsum at tick NT     (chain end)
    lnK = np.zeros((K, B), np.float64)   # final sink rows
    for core in range(NCORES):
        b0 = core * BPC
        for s in range(NS):
            res = r.results[core][f"res{s}"].astype(np.float64)  # [21, NCOL]
            idx = np.arange(PPS)
            g, c = idx // NCOL, idx % NCOL
            p = s * PPS + idx
            k, bl = p // BPC, p % BPC
            lnS[k, b0 + bl] = res[3 + g, c]
            lnM[k, b0 + bl] = res[9 + g, c]
            lnE[k, b0 + bl] = res[15 + g, c]
            lnK[k, b0 + bl] = res[18 + g, c]

    # ---- telescoping: log lam_k relative to the exact chain 0 ----
    loglam = np.zeros((K, B), np.float64)
    prev_end = lnM[0]                      # chain 0 boundary read at tick SEG
    for k in range(1, K):
        loglam[k] = loglam[k - 1] + lnS[k] - prev_end
        prev_end = lnE[k]

    kb = np.clip((lens - 1) // SEG, 0, K - 1)
    ar = np.arange(B)
    logZ = lnK[kb, ar] - loglam[kb, ar] + CSHIFT * lens.astype(np.float64)
    return (gp - logZ).astype(np.float32)
